# revision 1
# baseline (speedup 1.0000x reference)
"""Distributed Bass kernel for nn_Attention_12953621365048 (8 TRN2 NeuronCores).

Sharding: 2 batch-groups x 4 head-groups (3 heads/core).
  core c: batch b = c//4, heads 3*(c%4) .. 3*(c%4)+2
Per core: qkv/kv matmuls (transposed [dim, tok] layout), RMSNorm + RoPE,
attention with no-max softmax (scores bounded: q,k RMSNorm'd), 8-way AllToAll
(block-duplicated across batches) to turn head-sharding into token-sharding,
then a 24-tile projection against per-core batch-masked Wproj.
Host side only shards/gathers (transpose/concat/slice/zero-pad).
"""

from contextlib import ExitStack

import numpy as np
import ml_dtypes

import concourse.bass as bass
import concourse.mybir as mybir
import concourse.tile as tile
from concourse import bacc
from concourse.bass_utils import run_bass_kernel_spmd

B, N, M, C, H, HD, RD = 2, 2048, 512, 1536, 12, 128, 64
EPS = 1e-6
NHL = 3               # heads per core
S = N + M             # 2560 kv tokens
KT = S // 128         # 20 kv tiles
NQC = N // 512        # 4 q-chunks of 512 (== A2A block count per batch)
CH = 1024             # qkv-phase token chunk (bf16 moving limit)
F32 = mybir.dt.float32
F32R = mybir.dt.float32r
AF = mybir.ActivationFunctionType
ALU = mybir.AluOpType
BF16 = mybir.dt.bfloat16
NCT = C // 128        # 12 contraction tiles


def _r(ap):
    return ap.bitcast(F32R)


def build_nc():
    nc = bacc.Bacc("TRN2", target_bir_lowering=False, debug=False, num_devices=8)

    xT = nc.dram_tensor("xT", [C, N], BF16, kind="ExternalInput").ap()
    yT = nc.dram_tensor("yT", [C, M], BF16, kind="ExternalInput").ap()
    wqkv = nc.dram_tensor("wqkv", [C, 3 * NHL * HD], BF16, kind="ExternalInput").ap()
    wkv = nc.dram_tensor("wkv", [C, 2 * NHL * HD], BF16, kind="ExternalInput").ap()
    wproj = nc.dram_tensor("wproj", [2 * C, C], BF16, kind="ExternalInput").ap()
    wq = nc.dram_tensor("wq", [1, HD], F32, kind="ExternalInput").ap()
    wk = nc.dram_tensor("wk", [1, HD], F32, kind="ExternalInput").ap()
    cs = nc.dram_tensor("cs", [RD, N], BF16, kind="ExternalInput").ap()
    sn = nc.dram_tensor("sn", [RD, N], BF16, kind="ExternalInput").ap()
    ywT = nc.dram_tensor("ywT", [128, M // 128], F32, kind="ExternalInput").ap()
    bpr = nc.dram_tensor("bpr", [1, C], F32, kind="ExternalInput").ap()
    onesd = nc.dram_tensor("onesd", [128, 1], F32R, kind="ExternalInput").ap()
    onesb = nc.dram_tensor("onesb", [128, 1], BF16, kind="ExternalInput").ap()
    out = nc.dram_tensor("out", [512, C], F32, kind="ExternalOutput").ap()

    with tile.TileContext(nc) as tc, ExitStack() as ctx:
        # ---------- outer (whole-kernel) pools ----------
        pers = ctx.enter_context(tc.tile_pool(name="persist", bufs=1))
        dram = ctx.enter_context(tc.tile_pool(name="dram", bufs=1, space="DRAM"))

        ones_sb = pers.tile([128, 1], F32R, tag="ones")
        nc.sync.dma_start(ones_sb[:], onesd)
        onesb_sb = pers.tile([128, 1], BF16, tag="onesb")
        nc.sync.dma_start(onesb_sb[:], onesb)
        eps_sb = pers.tile([1, 1], F32, tag="eps")
        nc.vector.memset(eps_sb[:], EPS)
        wq_sb = pers.tile([128, 1], F32, tag="wq")
        nc.sync.dma_start(wq_sb[:], wq.rearrange("o p -> p o"))
        wk_sb = pers.tile([128, 1], F32, tag="wk")
        nc.sync.dma_start(wk_sb[:], wk.rearrange("o p -> p o"))

        # attention bias per kv tile column: 0 for x tokens, log(clip(w)) for y
        bias_sb = pers.tile([128, KT], F32, tag="bias")
        nc.vector.memset(bias_sb[:, 0 : N // 128], 0.0)
        ywT_sb = pers.tile([128, M // 128], F32, tag="ywT")
        nc.sync.dma_start(ywT_sb[:], ywT)
        ywc = pers.tile([128, M // 128], F32, tag="ywc")
        nc.vector.tensor_scalar_max(ywc[:], ywT_sb[:], 1e-4)
        nc.scalar.activation(bias_sb[:, N // 128 : KT], ywc[:], AF.Ln)

        # persistent activations
        qn = [pers.tile([128, N], BF16, tag=f"qn{t}", name=f"qn{t}") for t in range(NHL)]
        kn = [pers.tile([128, S], BF16, tag=f"kn{t}", name=f"kn{t}") for t in range(NHL)]
        v_sb = pers.tile([128, KT * NHL * HD], BF16, tag="v")  # [kv_tile, head, hd]

        outp = ctx.enter_context(tc.tile_pool(name="osb", bufs=2))
        a2a_in = dram.tile([2 * NQC, NHL, 128, 512], F32R)
        a2a_out = dram.tile([2 * NQC, NHL, 128, 512], F32R)

        # ---------- phase A/B: qkv + kv, norm, rope ----------
        with ExitStack() as ab:
            csn = ab.enter_context(tc.tile_pool(name="csn", bufs=1))
            wbig = ab.enter_context(tc.tile_pool(name="wbig", bufs=1))
            xtp = ab.enter_context(tc.tile_pool(name="xt", bufs=3))
            sqp = ab.enter_context(tc.tile_pool(name="sq", bufs=2))
            smallp = ab.enter_context(tc.tile_pool(name="small", bufs=3))
            brp = ab.enter_context(tc.tile_pool(name="bcast", bufs=2))
            ropep = ab.enter_context(tc.tile_pool(name="rope", bufs=2))
            psA = ab.enter_context(tc.tile_pool(name="psA", bufs=2, space="PSUM"))
            psV = ab.enter_context(tc.tile_pool(name="psV", bufs=2, space="PSUM"))
            psS = ab.enter_context(tc.tile_pool(name="psS", bufs=1, space="PSUM"))


            def norm_head(raw_ps, dst, w_sb, rope_q0, CHc):
                """RMSNorm over partition dim (HD) + optional RoPE; [128,CHc]."""
                sq = sqp.tile([128, CH], BF16, tag="sq", name="sq")[:, :CHc]
                nc.scalar.activation(sq, raw_ps[:], AF.Square)
                ssq = psS.tile([1, CH], F32, tag="ssq", name="ssq")[:, :CHc]
                for h0 in range(0, CHc, 512):
                    hw = min(512, CHc - h0)
                    nc.tensor.matmul(
                        ssq[:, h0 : h0 + hw],
                        onesb_sb[:],
                        sq[:, h0 : h0 + hw],
                        start=True,
                        stop=True,
                    )
                inv = smallp.tile([1, CH], F32, tag="inv", name="inv")[:, :CHc]
                nc.scalar.activation(
                    inv, ssq, AF.Abs_reciprocal_sqrt, bias=eps_sb[:],
                    scale=1.0 / HD,
                )
                binv = brp.tile([128, CH], F32, tag="binv", name="binv")[:, :CHc]
                nc.gpsimd.partition_broadcast(binv, inv)
                nc.vector.scalar_tensor_tensor(
                    dst, raw_ps[:], w_sb[:], binv, op0=ALU.mult, op1=ALU.mult
                )
                if rope_q0 is not None:
                    hf = RD // 2
                    csc = cs_sb[:, rope_q0 : rope_q0 + CHc]
                    snc = sn_sb[:, rope_q0 : rope_q0 + CHc]
                    sw = ropep.tile([RD, CH], BF16, tag="sw", name="sw")[:, :CHc]
                    nc.scalar.copy(sw[0:hf, :], dst[hf:RD, :])
                    nc.scalar.copy(sw[hf:RD, :], dst[0:hf, :])
                    ma = ropep.tile([RD, CH], BF16, tag="ma", name="ma")[:, :CHc]
                    mb = ropep.tile([RD, CH], BF16, tag="mb", name="mb")[:, :CHc]
                    nc.vector.tensor_mul(ma, dst[0:RD, :], csc)
                    nc.vector.tensor_mul(mb, sw, snc)
                    nc.vector.tensor_add(dst[0:RD, :], ma, mb)

            def qkv_chunk(src_sb, w_sb, nqh, q0, kdst_off, vt0, rope, CHc):
                """One CHc-token chunk: q (nqh heads), k (NHL heads), v (NHL heads)."""
                for t in range(nqh):
                    ps = psA.tile([128, CH], F32, tag="qk", name="qk")[:, :CHc]
                    for ct in range(NCT):
                        for h0 in range(0, CHc, 512):
                            hw = min(512, CHc - h0)
                            nc.tensor.matmul(
                                ps[:, h0 : h0 + hw],
                                w_sb[:, ct, t * HD : (t + 1) * HD],
                                src_sb[:, ct, h0 : h0 + hw],
                                start=(ct == 0),
                                stop=(ct == NCT - 1),
                            )
                    norm_head(
                        ps, qn[t][:, q0 : q0 + CHc], wq_sb,
                        q0 if rope else None, CHc,
                    )
                koff = nqh * HD
                for t in range(NHL):
                    ps = psA.tile([128, CH], F32, tag="qk", name="qk")[:, :CHc]
                    for ct in range(NCT):
                        for h0 in range(0, CHc, 512):
                            hw = min(512, CHc - h0)
                            nc.tensor.matmul(
                                ps[:, h0 : h0 + hw],
                                w_sb[:, ct, koff + t * HD : koff + (t + 1) * HD],
                                src_sb[:, ct, h0 : h0 + hw],
                                start=(ct == 0),
                                stop=(ct == NCT - 1),
                            )
                    norm_head(
                        ps,
                        kn[t][:, kdst_off : kdst_off + CHc],
                        wk_sb,
                        q0 if rope else None,
                        CHc,
                    )
                voff = (nqh + NHL) * HD
                for ts in range(CHc // 128):
                    ps = psV.tile([128, NHL * HD], F32, tag="vps")
                    for ct in range(NCT):
                        nc.tensor.matmul(
                            ps[:],
                            src_sb[:, ct, ts * 128 : (ts + 1) * 128],
                            w_sb[:, ct, voff : voff + NHL * HD],
                            start=(ct == 0),
                            stop=(ct == NCT - 1),
                        )
                    kvt = vt0 + ts
                    nc.vector.tensor_copy(
                        v_sb[:, kvt * NHL * HD : (kvt + 1) * NHL * HD], ps[:]
                    )

            xt_first = xtp.tile([128, NCT, CH], BF16, tag="xt", name="xt_first")
            wqkv_sb = wbig.tile([128, NCT, 3 * NHL * HD], BF16, tag="wbig")
            for ct in range(NCT):
                nc.sync.dma_start(
                    wqkv_sb[:, ct, :], wqkv[ct * 128 : (ct + 1) * 128, :]
                )
                nc.sync.dma_start(
                    xt_first[:, ct, :],
                    xT[ct * 128 : (ct + 1) * 128, 0:CH],
                )
            cs_sb = csn.tile([RD, N], BF16, tag="cs")
            nc.sync.dma_start(cs_sb[:], cs)
            sn_sb = csn.tile([RD, N], BF16, tag="sn")
            nc.sync.dma_start(sn_sb[:], sn)
            for qc in range(N // CH):
                q0 = qc * CH
                if qc == 0:
                    xt_sb = xt_first
                else:
                    xt_sb = xtp.tile([128, NCT, CH], BF16, tag="xt")
                    nc.sync.dma_start(
                        xt_sb[:],
                        xT[:, q0 : q0 + CH].rearrange("(ct p) q -> p ct q", p=128),
                    )
                qkv_chunk(xt_sb, wqkv_sb, NHL, q0, q0, q0 // 128, rope=True, CHc=CH)

            wkv_sb = wbig.tile([128, NCT, 3 * NHL * HD], BF16, tag="wbig")
            for ct in range(NCT):
                nc.sync.dma_start(
                    wkv_sb[:, ct, : 2 * NHL * HD], wkv[ct * 128 : (ct + 1) * 128, :]
                )
            yt_sb = xtp.tile([128, NCT, CH], BF16, tag="xt")
            nc.sync.dma_start(
                yt_sb[:, :, :M], yT.rearrange("(ct p) q -> p ct q", p=128)
            )
            qkv_chunk(yt_sb, wkv_sb, 0, 0, N, N // 128, rope=False, CHc=M)

        # ---------- phase C: attention + per-head A2A + interleaved proj ----------
        with ExitStack() as pc:
            expp = pc.enter_context(tc.tile_pool(name="exp", bufs=6))
            exsp = pc.enter_context(tc.tile_pool(name="exs", bufs=3))
            brp2 = pc.enter_context(tc.tile_pool(name="bcast2", bufs=2))
            smallc = pc.enter_context(tc.tile_pool(name="smallc", bufs=2))
            accp = pc.enter_context(tc.tile_pool(name="accp", bufs=1))
            pjp = pc.enter_context(tc.tile_pool(name="pjp", bufs=1))
            wpre = pc.enter_context(tc.tile_pool(name="wpre", bufs=2))
            psSc = pc.enter_context(tc.tile_pool(name="psSc", bufs=2, space="PSUM"))
            psAv = pc.enter_context(tc.tile_pool(name="psAv", bufs=1, space="PSUM"))
            psDen = pc.enter_context(tc.tile_pool(name="psDen", bufs=1, space="PSUM"))
            psP = pc.enter_context(tc.tile_pool(name="psP", bufs=1, space="PSUM"))

            bpr_sb = pjp.tile([1, C], F32, tag="bpr")
            nc.sync.dma_start(bpr_sb[:], bpr)
            bb_sb = pjp.tile([128, C], F32, tag="bb")
            nc.gpsimd.partition_broadcast(bb_sb[:], bpr_sb[:])

            a2a_ins = [
                dram.tile([2 * NQC, 128, 512], BF16, name=f"a2ai{t}") for t in range(NHL)
            ]
            a2a_outs = [
                dram.tile([2 * NQC, 128, 512], BF16, name=f"a2ao{t}") for t in range(NHL)
            ]
            acc = [
                accp.tile([128, 512], F32, tag=f"acc{i}", name=f"acc{i}")
                for i in range(12)
            ]

            def prefetch_w(t):
                wp = wpre.tile([128, 24, 512], BF16, tag="wpre", name=f"wpre{t}")
                for i in range(2 * NQC):
                    nc.sync.dma_start(
                        wp[:, 3 * i : 3 * (i + 1), :],
                        wproj[t * 1024 + i * 128 : t * 1024 + (i + 1) * 128, :],
                    )
                return wp

            def attention_head(t, after_chunk0=None):
                for qc in range(NQC):
                    if qc == 1 and after_chunk0 is not None:
                        after_chunk0()
                    av = psAv.tile([128, 512], F32, tag="av")
                    den = psDen.tile([1, 512], F32, tag="den")
                    pair_exs = []
                    quad_exs = []
                    for kp in range(KT // 2):
                        sc = psSc.tile([128, 1024], F32, tag="sc")
                        for kh in range(2):
                            kt = 2 * kp + kh
                            nc.tensor.matmul(
                                sc[:, kh * 512 : (kh + 1) * 512],
                                kn[t][:, kt * 128 : (kt + 1) * 128],
                                qn[t][:, qc * 512 : (qc + 1) * 512],
                                start=True,
                                stop=True,
                            )
                        ex = expp.tile([128, 1024], BF16, tag="ex")
                        if kp < 8:
                            nc.scalar.activation(
                                ex[:], sc[:], AF.Exp, bias=bias_sb[:, 0:1]
                            )
                        else:
                            for kh in range(2):
                                kt = 2 * kp + kh
                                nc.scalar.activation(
                                    ex[:, kh * 512 : (kh + 1) * 512],
                                    sc[:, kh * 512 : (kh + 1) * 512],
                                    AF.Exp,
                                    bias=bias_sb[:, kt : kt + 1],
                                )
                        for kh in range(2):
                            kt = 2 * kp + kh
                            nc.tensor.matmul(
                                av[:],
                                v_sb[
                                    :,
                                    kt * NHL * HD
                                    + t * HD : kt * NHL * HD
                                    + (t + 1) * HD,
                                ],
                                ex[:, kh * 512 : (kh + 1) * 512],
                                start=(kt == 0),
                                stop=(kt == KT - 1),
                            )
                        exs = exsp.tile([128, 512], BF16, tag="exs", bufs=4)
                        nc.vector.tensor_add(
                            exs[:], ex[:, 0:512], ex[:, 512:1024]
                        )
                        pair_exs.append(exs)
                        if len(pair_exs) == 2:
                            exq = exsp.tile([128, 512], BF16, tag="exq", bufs=3)
                            nc.vector.tensor_add(
                                exq[:], pair_exs[0][:], pair_exs[1][:]
                            )
                            pair_exs.clear()
                            quad_exs.append(exq)
                        if kp == KT // 2 - 1 and pair_exs:
                            quad_exs.append(pair_exs.pop())
                        if len(quad_exs) == 2 or (kp == KT // 2 - 1 and quad_exs):
                            if len(quad_exs) == 2:
                                exo = exsp.tile([128, 512], BF16, tag="exo", bufs=2)
                                nc.vector.tensor_add(
                                    exo[:], quad_exs[0][:], quad_exs[1][:]
                                )
                            else:
                                exo = quad_exs[0]
                            nc.tensor.matmul(
                                den[:],
                                onesb_sb[:],
                                exo[:],
                                start=(kp <= 3),
                                stop=(kp == KT // 2 - 1),
                            )
                            quad_exs.clear()
                    av_s = smallc.tile([128, 512], F32, tag="av_s", bufs=2)
                    nc.vector.tensor_copy(av_s[:], av[:])
                    invd = smallc.tile([1, 512], F32, tag="invd")
                    nc.vector.reciprocal(invd[:], den[:])
                    bden = brp2.tile([128, 512], F32, tag="bden")
                    nc.gpsimd.partition_broadcast(bden[:], invd[:])
                    o_sb = outp.tile([128, 512], BF16, tag="o")
                    nc.vector.tensor_mul(o_sb[:], av_s[:], bden[:])
                    nc.sync.dma_start(a2a_ins[t][qc], o_sb[:])
                    nc.sync.dma_start(a2a_ins[t][NQC + qc], o_sb[:])

            def a2a_head(t):
                nc.gpsimd.collective_compute(
                    "AllToAll",
                    ALU.bypass,
                    replica_groups=[[0, 1, 2, 3, 4, 5, 6, 7]],
                    ins=[a2a_ins[t].opt()],
                    outs=[a2a_outs[t].opt()],
                )

            def proj_partial(t, wp):
                pj_t = pjp.tile([128, 2 * NQC, 512], BF16, tag=f"pj{t}", name=f"pj{t}")
                nc.sync.dma_start(pj_t[:], a2a_outs[t].rearrange("i p q -> p i q"))
                for fc in range(3):
                    for th in range(2):
                        pps = [
                            psP.tile(
                                [128, 512], F32, tag=f"pp{tp_}",
                                name=f"pp{t}_{fc}_{th}_{tp_}",
                            )
                            for tp_ in range(2)
                        ]
                        for i in range(2 * NQC):
                            for tp_ in range(2):
                                tcc = th * 2 + tp_
                                nc.tensor.matmul(
                                    pps[tp_][:],
                                    pj_t[:, i, tcc * 128 : (tcc + 1) * 128],
                                    wp[:, 3 * i + fc, :],
                                    start=(i == 0),
                                    stop=(i == 2 * NQC - 1),
                                )
                        for tp_ in range(2):
                            tcc = th * 2 + tp_
                            a = acc[fc * 4 + tcc]
                            if t == 0:
                                nc.vector.tensor_copy(a[:], pps[tp_][:])
                            else:
                                nc.vector.tensor_add(a[:], a[:], pps[tp_][:])

            wp0 = prefetch_w(0)
            attention_head(0)
            wp1 = prefetch_w(1)
            attention_head(1, after_chunk0=lambda: a2a_head(0))
            proj_partial(0, wp0)
            wp2 = prefetch_w(2)
            attention_head(2, after_chunk0=lambda: a2a_head(1))
            proj_partial(1, wp1)
            a2a_head(2)
            proj_partial(2, wp2)

            # epilogue: bias + store
            for fc in range(3):
                for tcc in range(4):
                    ob = outp.tile([128, 512], F32, tag="ob")
                    nc.vector.tensor_tensor(
                        ob[:],
                        acc[fc * 4 + tcc][:],
                        bb_sb[:, fc * 512 : (fc + 1) * 512],
                        ALU.add,
                    )
                    nc.sync.dma_start(
                        out[tcc * 128 : (tcc + 1) * 128, fc * 512 : (fc + 1) * 512],
                        ob[:],
                    )
    nc.compile()
    return nc


_NC_CACHE = {}


def _get_nc():
    if "nc" not in _NC_CACHE:
        _NC_CACHE["nc"] = build_nc()
    return _NC_CACHE["nc"]


def make_in_maps(x, y, pos, y_token_weights, Wqkv, Wkv, q_norm_w, k_norm_w, Wproj, bproj):
    f = np.float32
    c32 = pos[:, :, 0].T
    s32 = pos[:, :, 1].T
    csT = np.ascontiguousarray(
        np.concatenate([c32, c32], 0).astype(ml_dtypes.bfloat16))   # [64, N]
    snT = np.ascontiguousarray(
        np.concatenate([-s32, s32], 0).astype(ml_dtypes.bfloat16))  # [64, N]
    wqs = (np.asarray(q_norm_w, dtype=f) * np.float32(HD) ** -0.5).reshape(1, HD)
    wkk = np.asarray(k_norm_w, dtype=f).reshape(1, HD)
    Wp = np.asarray(Wproj, dtype=f)
    wproj24 = []
    for b in range(B):
        W = np.zeros((NHL, 8, 128, C), dtype=f)
        for i in range(8):
            if i // 4 == b:
                for t in range(NHL):
                    h = 3 * (i % 4) + t
                    W[t, i] = Wp[h * 128 : (h + 1) * 128, :]
        wproj24.append(
            np.ascontiguousarray(W.reshape(NHL * 8 * 128, C).astype(ml_dtypes.bfloat16))
        )
    in_maps = []
    for c in range(8):
        b, g = c // 4, c % 4
        heads = [3 * g + i for i in range(NHL)]
        qcols = [Wqkv[:, h * HD : (h + 1) * HD] for h in heads]
        kcols = [Wqkv[:, C + h * HD : C + (h + 1) * HD] for h in heads]
        vcols = [Wqkv[:, 2 * C + h * HD : 2 * C + (h + 1) * HD] for h in heads]
        wqkv_c = np.ascontiguousarray(
            np.concatenate(qcols + kcols + vcols, axis=1), dtype=f
        )
        kcols2 = [Wkv[:, h * HD : (h + 1) * HD] for h in heads]
        vcols2 = [Wkv[:, C + h * HD : C + (h + 1) * HD] for h in heads]
        wkv_c = np.ascontiguousarray(np.concatenate(kcols2 + vcols2, axis=1), dtype=f)
        in_maps.append(
            {
                "xT": np.ascontiguousarray(np.asarray(x)[b].T.astype(ml_dtypes.bfloat16)),
                "yT": np.ascontiguousarray(np.asarray(y)[b].T.astype(ml_dtypes.bfloat16)),
                "wqkv": wqkv_c.astype(ml_dtypes.bfloat16),
                "wkv": wkv_c.astype(ml_dtypes.bfloat16),
                "wproj": wproj24[b],
                "wq": np.ascontiguousarray(wqs),
                "wk": np.ascontiguousarray(wkk),
                "cs": csT,
                "sn": snT,
                "ywT": np.ascontiguousarray(
                    np.asarray(y_token_weights)[b].reshape(M // 128, 128).T, dtype=f
                ),
                "bpr": np.asarray(bproj, dtype=f).reshape(1, C),
                "onesd": np.ones((128, 1), dtype=f),
                "onesb": np.ones((128, 1), dtype=ml_dtypes.bfloat16),
            }
        )
    return in_maps


def kernel(x, y, pos, y_token_weights, Wqkv, Wkv, q_norm_w, k_norm_w, Wproj, bproj,
           _trace=False):
    x = np.asarray(x, dtype=np.float32)
    y = np.asarray(y, dtype=np.float32)
    pos = np.asarray(pos, dtype=np.float32)
    y_token_weights = np.asarray(y_token_weights, dtype=np.float32)
    nc = _get_nc()
    in_maps = make_in_maps(
        x, y, pos, y_token_weights,
        np.asarray(Wqkv), np.asarray(Wkv), np.asarray(q_norm_w),
        np.asarray(k_norm_w), np.asarray(Wproj), np.asarray(bproj),
    )
    res = run_bass_kernel_spmd(nc, in_maps, core_ids=list(range(8)), trace=_trace)
    outp = np.zeros((B, N, C), dtype=np.float32)
    for c in range(8):
        b, g = c // 4, c % 4
        outp[b, g * 512 : (g + 1) * 512, :] = res.results[c]["out"]
    if _trace:
        return outp, res
    return outp



# revision 12
# speedup vs baseline: 1.1898x; 1.1898x over previous
"""Distributed Bass kernel for nn_Attention_12953621365048 (8 TRN2 NeuronCores).

Sharding: 2 batch-groups x 4 head-groups (3 heads/core).
  core c: batch b = c//4, heads 3*(c%4) .. 3*(c%4)+2
Per core: qkv/kv matmuls (transposed [dim, tok] layout), RMSNorm + RoPE,
attention with no-max softmax (scores bounded: q,k RMSNorm'd), 8-way AllToAll
(wrong-batch duplicate blocks zeroed via per-core m0/m1 masks) to turn
head-sharding into token-sharding; receiver folds the two batch halves and
runs a 12-tile projection against head-permuted Wproj.
Softmax denominator is broadcast via a K=1 matmul + fast reciprocal.
Host side only shards/gathers (transpose/concat/slice/zero-pad).
"""

from contextlib import ExitStack

import numpy as np
import ml_dtypes

import concourse.bass as bass
import concourse.mybir as mybir
import concourse.tile as tile
from concourse import bacc
from concourse.bass_utils import run_bass_kernel_spmd

B, N, M, C, H, HD, RD = 2, 2048, 512, 1536, 12, 128, 64
EPS = 1e-6
NHL = 3               # heads per core
S = N + M             # 2560 kv tokens
KT = S // 128         # 20 kv tiles
NQC = N // 512        # 4 q-chunks of 512 (== A2A block count per batch)
CH = 1024             # qkv-phase token chunk (bf16 moving limit)
F32 = mybir.dt.float32
F32R = mybir.dt.float32r
AF = mybir.ActivationFunctionType
ALU = mybir.AluOpType
BF16 = mybir.dt.bfloat16
NCT = C // 128        # 12 contraction tiles


def _r(ap):
    return ap.bitcast(F32R)


def build_nc():
    nc = bacc.Bacc("TRN2", target_bir_lowering=False, debug=False, num_devices=8)

    xT = nc.dram_tensor("xT", [C, N], BF16, kind="ExternalInput").ap()
    yT = nc.dram_tensor("yT", [C, M], BF16, kind="ExternalInput").ap()
    wqkv = nc.dram_tensor("wqkv", [C, 3 * NHL * HD], BF16, kind="ExternalInput").ap()
    wkv = nc.dram_tensor("wkv", [C, 2 * NHL * HD], BF16, kind="ExternalInput").ap()
    wproj = nc.dram_tensor("wproj", [C, C], BF16, kind="ExternalInput").ap()
    wq = nc.dram_tensor("wq", [1, HD], F32, kind="ExternalInput").ap()
    wk = nc.dram_tensor("wk", [1, HD], F32, kind="ExternalInput").ap()
    cs = nc.dram_tensor("cs", [RD, N], BF16, kind="ExternalInput").ap()
    sn = nc.dram_tensor("sn", [RD, N], BF16, kind="ExternalInput").ap()
    ywT = nc.dram_tensor("ywT", [128, M // 128], F32, kind="ExternalInput").ap()
    bpr = nc.dram_tensor("bpr", [1, C], F32, kind="ExternalInput").ap()
    onesd = nc.dram_tensor("onesd", [128, 1], F32R, kind="ExternalInput").ap()
    onesb = nc.dram_tensor("onesb", [128, 1], BF16, kind="ExternalInput").ap()
    ones128 = nc.dram_tensor("ones128", [1, 128], F32R, kind="ExternalInput").ap()
    m0d = nc.dram_tensor("m0d", [128, 1], F32, kind="ExternalInput").ap()
    m1d = nc.dram_tensor("m1d", [128, 1], F32, kind="ExternalInput").ap()
    out = nc.dram_tensor("out", [512, C], F32, kind="ExternalOutput").ap()

    with tile.TileContext(nc) as tc, ExitStack() as ctx:
        # ---------- outer (whole-kernel) pools ----------
        pers = ctx.enter_context(tc.tile_pool(name="persist", bufs=1))
        dram = ctx.enter_context(tc.tile_pool(name="dram", bufs=1, space="DRAM"))

        ones_sb = pers.tile([128, 1], F32R, tag="ones")
        nc.sync.dma_start(ones_sb[:], onesd)
        onesb_sb = pers.tile([128, 1], BF16, tag="onesb")
        nc.sync.dma_start(onesb_sb[:], onesb)
        ones128_sb = pers.tile([1, 128], F32R, tag="ones128")
        nc.sync.dma_start(ones128_sb[:], ones128)
        m0_sb = pers.tile([128, 1], F32, tag="m0")
        nc.sync.dma_start(m0_sb[:], m0d)
        m1_sb = pers.tile([128, 1], F32, tag="m1")
        nc.sync.dma_start(m1_sb[:], m1d)
        eps_sb = pers.tile([1, 1], F32, tag="eps")
        nc.vector.memset(eps_sb[:], EPS)
        wq_sb = pers.tile([128, 1], F32, tag="wq")
        nc.sync.dma_start(wq_sb[:], wq.rearrange("o p -> p o"))
        wk_sb = pers.tile([128, 1], F32, tag="wk")
        nc.sync.dma_start(wk_sb[:], wk.rearrange("o p -> p o"))

        # attention bias per kv tile column: 0 for x tokens, log(clip(w)) for y
        bias_sb = pers.tile([128, KT], F32, tag="bias")
        nc.vector.memset(bias_sb[:, 0 : N // 128], 0.0)
        ywT_sb = pers.tile([128, M // 128], F32, tag="ywT")
        nc.sync.dma_start(ywT_sb[:], ywT)
        ywc = pers.tile([128, M // 128], F32, tag="ywc")
        nc.vector.tensor_scalar_max(ywc[:], ywT_sb[:], 1e-4)
        nc.scalar.activation(bias_sb[:, N // 128 : KT], ywc[:], AF.Ln)

        # persistent activations
        qn = [pers.tile([128, N], BF16, tag=f"qn{t}", name=f"qn{t}") for t in range(NHL)]
        kn = [pers.tile([128, S], BF16, tag=f"kn{t}", name=f"kn{t}") for t in range(NHL)]
        v_sb = pers.tile([128, KT * NHL * HD], BF16, tag="v")  # [kv_tile, head, hd]

        outp = ctx.enter_context(tc.tile_pool(name="osb", bufs=2))
        a2a_in = dram.tile([2 * NQC, NHL, 128, 512], F32R)
        a2a_out = dram.tile([2 * NQC, NHL, 128, 512], F32R)

        # ---------- phase A/B: qkv + kv, norm, rope ----------
        with ExitStack() as ab:
            csn = ab.enter_context(tc.tile_pool(name="csn", bufs=1))
            wbig = ab.enter_context(tc.tile_pool(name="wbig", bufs=1))
            xtp = ab.enter_context(tc.tile_pool(name="xt", bufs=3))
            sqp = ab.enter_context(tc.tile_pool(name="sq", bufs=2))
            smallp = ab.enter_context(tc.tile_pool(name="small", bufs=3))
            brp = ab.enter_context(tc.tile_pool(name="bcast", bufs=2))
            ropep = ab.enter_context(tc.tile_pool(name="rope", bufs=2))
            psA = ab.enter_context(tc.tile_pool(name="psA", bufs=2, space="PSUM"))
            psV = ab.enter_context(tc.tile_pool(name="psV", bufs=2, space="PSUM"))
            psS = ab.enter_context(tc.tile_pool(name="psS", bufs=1, space="PSUM"))


            def norm_head(raw_ps, dst, w_sb, rope_q0, CHc):
                """RMSNorm over partition dim (HD) + optional RoPE; [128,CHc]."""
                sq = sqp.tile([128, CH], BF16, tag="sq", name="sq")[:, :CHc]
                nc.scalar.activation(sq, raw_ps[:], AF.Square)
                ssq = psS.tile([1, CH], F32, tag="ssq", name="ssq")[:, :CHc]
                for h0 in range(0, CHc, 512):
                    hw = min(512, CHc - h0)
                    nc.tensor.matmul(
                        ssq[:, h0 : h0 + hw],
                        onesb_sb[:],
                        sq[:, h0 : h0 + hw],
                        start=True,
                        stop=True,
                    )
                inv = smallp.tile([1, CH], F32, tag="inv", name="inv")[:, :CHc]
                nc.scalar.activation(
                    inv, ssq, AF.Abs_reciprocal_sqrt, bias=eps_sb[:],
                    scale=1.0 / HD,
                )
                binv = brp.tile([128, CH], F32, tag="binv", name="binv")[:, :CHc]
                nc.gpsimd.partition_broadcast(binv, inv)
                nc.vector.scalar_tensor_tensor(
                    dst, raw_ps[:], w_sb[:], binv, op0=ALU.mult, op1=ALU.mult
                )
                if rope_q0 is not None:
                    hf = RD // 2
                    csc = cs_sb[:, rope_q0 : rope_q0 + CHc]
                    snc = sn_sb[:, rope_q0 : rope_q0 + CHc]
                    sw = ropep.tile([RD, CH], BF16, tag="sw", name="sw")[:, :CHc]
                    nc.scalar.copy(sw[0:hf, :], dst[hf:RD, :])
                    nc.scalar.copy(sw[hf:RD, :], dst[0:hf, :])
                    ma = ropep.tile([RD, CH], BF16, tag="ma", name="ma")[:, :CHc]
                    mb = ropep.tile([RD, CH], BF16, tag="mb", name="mb")[:, :CHc]
                    nc.vector.tensor_mul(ma, dst[0:RD, :], csc)
                    nc.vector.tensor_mul(mb, sw, snc)
                    nc.vector.tensor_add(dst[0:RD, :], ma, mb)

            def qkv_chunk(src_sb, w_sb, nqh, q0, kdst_off, vt0, rope, CHc):
                """One CHc-token chunk: q (nqh heads), k (NHL heads), v (NHL heads)."""
                for t in range(nqh):
                    ps = psA.tile([128, CH], F32, tag="qk", name="qk")[:, :CHc]
                    for ct in range(NCT):
                        for h0 in range(0, CHc, 512):
                            hw = min(512, CHc - h0)
                            nc.tensor.matmul(
                                ps[:, h0 : h0 + hw],
                                w_sb[:, ct, t * HD : (t + 1) * HD],
                                src_sb[:, ct, h0 : h0 + hw],
                                start=(ct == 0),
                                stop=(ct == NCT - 1),
                            )
                    norm_head(
                        ps, qn[t][:, q0 : q0 + CHc], wq_sb,
                        q0 if rope else None, CHc,
                    )
                koff = nqh * HD
                for t in range(NHL):
                    ps = psA.tile([128, CH], F32, tag="qk", name="qk")[:, :CHc]
                    for ct in range(NCT):
                        for h0 in range(0, CHc, 512):
                            hw = min(512, CHc - h0)
                            nc.tensor.matmul(
                                ps[:, h0 : h0 + hw],
                                w_sb[:, ct, koff + t * HD : koff + (t + 1) * HD],
                                src_sb[:, ct, h0 : h0 + hw],
                                start=(ct == 0),
                                stop=(ct == NCT - 1),
                            )
                    norm_head(
                        ps,
                        kn[t][:, kdst_off : kdst_off + CHc],
                        wk_sb,
                        q0 if rope else None,
                        CHc,
                    )
                voff = (nqh + NHL) * HD
                for ts in range(CHc // 128):
                    ps = psV.tile([128, NHL * HD], F32, tag="vps")
                    for ct in range(NCT):
                        nc.tensor.matmul(
                            ps[:],
                            src_sb[:, ct, ts * 128 : (ts + 1) * 128],
                            w_sb[:, ct, voff : voff + NHL * HD],
                            start=(ct == 0),
                            stop=(ct == NCT - 1),
                        )
                    kvt = vt0 + ts
                    nc.vector.tensor_copy(
                        v_sb[:, kvt * NHL * HD : (kvt + 1) * NHL * HD], ps[:]
                    )

            xt_first = xtp.tile([128, NCT, CH], BF16, tag="xt", name="xt_first")
            wqkv_sb = wbig.tile([128, NCT, 3 * NHL * HD], BF16, tag="wbig")
            for ct in range(NCT):
                nc.sync.dma_start(
                    wqkv_sb[:, ct, :], wqkv[ct * 128 : (ct + 1) * 128, :]
                )
                nc.sync.dma_start(
                    xt_first[:, ct, :],
                    xT[ct * 128 : (ct + 1) * 128, 0:CH],
                )
            cs_sb = csn.tile([RD, N], BF16, tag="cs")
            nc.sync.dma_start(cs_sb[:], cs)
            sn_sb = csn.tile([RD, N], BF16, tag="sn")
            nc.sync.dma_start(sn_sb[:], sn)
            for qc in range(N // CH):
                q0 = qc * CH
                if qc == 0:
                    xt_sb = xt_first
                else:
                    xt_sb = xtp.tile([128, NCT, CH], BF16, tag="xt")
                    nc.sync.dma_start(
                        xt_sb[:],
                        xT[:, q0 : q0 + CH].rearrange("(ct p) q -> p ct q", p=128),
                    )
                qkv_chunk(xt_sb, wqkv_sb, NHL, q0, q0, q0 // 128, rope=True, CHc=CH)

            wkv_sb = wbig.tile([128, NCT, 3 * NHL * HD], BF16, tag="wbig")
            for ct in range(NCT):
                nc.sync.dma_start(
                    wkv_sb[:, ct, : 2 * NHL * HD], wkv[ct * 128 : (ct + 1) * 128, :]
                )
            yt_sb = xtp.tile([128, NCT, CH], BF16, tag="xt")
            nc.sync.dma_start(
                yt_sb[:, :, :M], yT.rearrange("(ct p) q -> p ct q", p=128)
            )
            qkv_chunk(yt_sb, wkv_sb, 0, 0, N, N // 128, rope=False, CHc=M)

        # ---------- phase C: attention + per-head A2A + interleaved proj ----------
        with ExitStack() as pc:
            expp = pc.enter_context(tc.tile_pool(name="exp", bufs=6))
            exsp = pc.enter_context(tc.tile_pool(name="exs", bufs=3))
            brp2 = pc.enter_context(tc.tile_pool(name="bcast2", bufs=2))
            smallc = pc.enter_context(tc.tile_pool(name="smallc", bufs=2))
            accp = pc.enter_context(tc.tile_pool(name="accp", bufs=1))
            pjp = pc.enter_context(tc.tile_pool(name="pjp", bufs=1))
            wpre = pc.enter_context(tc.tile_pool(name="wpre", bufs=2))
            psSc = pc.enter_context(tc.tile_pool(name="psSc", bufs=2, space="PSUM"))
            psAv = pc.enter_context(tc.tile_pool(name="psAv", bufs=1, space="PSUM"))
            psDen = pc.enter_context(tc.tile_pool(name="psDen", bufs=1, space="PSUM"))
            psP = pc.enter_context(tc.tile_pool(name="psP", bufs=1, space="PSUM"))

            bpr_sb = pjp.tile([1, C], F32, tag="bpr")
            nc.sync.dma_start(bpr_sb[:], bpr)
            bb_sb = pjp.tile([128, C], F32, tag="bb")
            nc.gpsimd.partition_broadcast(bb_sb[:], bpr_sb[:])

            a2a_ins = [
                dram.tile([2 * NQC, 128, 512], BF16, name=f"a2ai{t}") for t in range(NHL)
            ]
            a2a_outs = [
                dram.tile([2 * NQC, 128, 512], BF16, name=f"a2ao{t}") for t in range(NHL)
            ]
            acc = [
                accp.tile([128, 512], F32, tag=f"acc{i}", name=f"acc{i}")
                for i in range(12)
            ]

            def prefetch_w(t):
                wp = wpre.tile([128, 12, 512], BF16, tag="wpre", name=f"wpre{t}")
                for i in range(NQC):
                    nc.sync.dma_start(
                        wp[:, 3 * i : 3 * (i + 1), :],
                        wproj[t * 512 + i * 128 : t * 512 + (i + 1) * 128, :],
                    )
                return wp

            def attention_head(t, after_chunk0=None):
                for qc in range(NQC):
                    if qc == 1 and after_chunk0 is not None:
                        after_chunk0()
                    av = psAv.tile([128, 512], F32, tag="av")
                    den = psDen.tile([1, 512], F32, tag="den")
                    pair_exs = []
                    quad_exs = []
                    for kp in range(KT // 2):
                        sc = psSc.tile([128, 1024], F32, tag="sc")
                        for kh in range(2):
                            kt = 2 * kp + kh
                            nc.tensor.matmul(
                                sc[:, kh * 512 : (kh + 1) * 512],
                                kn[t][:, kt * 128 : (kt + 1) * 128],
                                qn[t][:, qc * 512 : (qc + 1) * 512],
                                start=True,
                                stop=True,
                            )
                        ex = expp.tile([128, 1024], BF16, tag="ex")
                        if kp < 8:
                            nc.scalar.activation(
                                ex[:], sc[:], AF.Exp, bias=bias_sb[:, 0:1]
                            )
                        else:
                            for kh in range(2):
                                kt = 2 * kp + kh
                                nc.scalar.activation(
                                    ex[:, kh * 512 : (kh + 1) * 512],
                                    sc[:, kh * 512 : (kh + 1) * 512],
                                    AF.Exp,
                                    bias=bias_sb[:, kt : kt + 1],
                                )
                        for kh in range(2):
                            kt = 2 * kp + kh
                            nc.tensor.matmul(
                                av[:],
                                v_sb[
                                    :,
                                    kt * NHL * HD
                                    + t * HD : kt * NHL * HD
                                    + (t + 1) * HD,
                                ],
                                ex[:, kh * 512 : (kh + 1) * 512],
                                start=(kt == 0),
                                stop=(kt == KT - 1),
                            )
                        exs = exsp.tile([128, 512], BF16, tag="exs", bufs=4)
                        nc.vector.tensor_add(
                            exs[:], ex[:, 0:512], ex[:, 512:1024]
                        )
                        pair_exs.append(exs)
                        if len(pair_exs) == 2:
                            exq = exsp.tile([128, 512], BF16, tag="exq", bufs=3)
                            nc.vector.tensor_add(
                                exq[:], pair_exs[0][:], pair_exs[1][:]
                            )
                            pair_exs.clear()
                            quad_exs.append(exq)
                        if kp == KT // 2 - 1 and pair_exs:
                            quad_exs.append(pair_exs.pop())
                        if len(quad_exs) == 2 or (kp == KT // 2 - 1 and quad_exs):
                            if len(quad_exs) == 2:
                                exo = exsp.tile([128, 512], BF16, tag="exo", bufs=2)
                                nc.vector.tensor_add(
                                    exo[:], quad_exs[0][:], quad_exs[1][:]
                                )
                            else:
                                exo = quad_exs[0]
                            nc.tensor.matmul(
                                den[:],
                                onesb_sb[:],
                                exo[:],
                                start=(kp <= 3),
                                stop=(kp == KT // 2 - 1),
                            )
                            quad_exs.clear()
                    av_s = smallc.tile([128, 512], F32, tag="av_s", bufs=2)
                    nc.vector.tensor_copy(av_s[:], av[:])
                    den_sb = smallc.tile([1, 512], F32R, tag="den_sb", bufs=2)
                    nc.vector.tensor_copy(den_sb[:], den[:])
                    # broadcast den across partitions via K=1 matmul (reuses av bank)
                    bden = psAv.tile([128, 512], F32, tag="av")
                    nc.tensor.matmul(
                        bden[:], ones128_sb[:], den_sb[:], start=True, stop=True
                    )
                    binv = brp2.tile([128, 512], F32, tag="binv")
                    nc.vector.reciprocal_approx_fast(binv[:], bden[:])
                    # o1/o2: per-core batch masks m0/m1 zero the wrong-batch copy
                    o1 = outp.tile([128, 512], BF16, tag="o")
                    nc.vector.scalar_tensor_tensor(
                        o1[:], av_s[:], m0_sb[:], binv[:], op0=ALU.mult, op1=ALU.mult
                    )
                    o2 = outp.tile([128, 512], BF16, tag="o")
                    nc.vector.scalar_tensor_tensor(
                        o2[:], av_s[:], m1_sb[:], binv[:], op0=ALU.mult, op1=ALU.mult
                    )
                    nc.sync.dma_start(a2a_ins[t][qc], o1[:])
                    nc.sync.dma_start(a2a_ins[t][NQC + qc], o2[:])

            def a2a_head(t):
                nc.gpsimd.collective_compute(
                    "AllToAll",
                    ALU.bypass,
                    replica_groups=[[0, 1, 2, 3, 4, 5, 6, 7]],
                    ins=[a2a_ins[t].opt()],
                    outs=[a2a_outs[t].opt()],
                )

            def proj_partial(t, wp):
                pj_t = pjp.tile([128, 2 * NQC, 512], BF16, tag=f"pj{t}", name=f"pj{t}")
                nc.sync.dma_start(pj_t[:], a2a_outs[t].rearrange("i p q -> p i q"))
                # fold the two batch halves (wrong-batch blocks are zeroed by
                # the sender masks) so the proj contraction is 4 blocks not 8
                pjf = pjp.tile([128, NQC, 512], BF16, tag=f"pjf{t}", name=f"pjf{t}")
                nc.vector.tensor_add(
                    pjf[:], pj_t[:, 0:NQC, :], pj_t[:, NQC : 2 * NQC, :]
                )
                for fc in range(3):
                    for th in range(2):
                        pps = [
                            psP.tile(
                                [128, 512], F32, tag=f"pp{tp_}",
                                name=f"pp{t}_{fc}_{th}_{tp_}",
                            )
                            for tp_ in range(2)
                        ]
                        for i in range(NQC):
                            for tp_ in range(2):
                                tcc = th * 2 + tp_
                                nc.tensor.matmul(
                                    pps[tp_][:],
                                    pjf[:, i, tcc * 128 : (tcc + 1) * 128],
                                    wp[:, 3 * i + fc, :],
                                    start=(i == 0),
                                    stop=(i == NQC - 1),
                                )
                        for tp_ in range(2):
                            tcc = th * 2 + tp_
                            a = acc[fc * 4 + tcc]
                            if t == 0:
                                nc.vector.tensor_copy(a[:], pps[tp_][:])
                            else:
                                nc.vector.tensor_add(a[:], a[:], pps[tp_][:])

            wp0 = prefetch_w(0)
            attention_head(0)
            wp1 = prefetch_w(1)
            attention_head(1, after_chunk0=lambda: a2a_head(0))
            proj_partial(0, wp0)
            wp2 = prefetch_w(2)
            attention_head(2, after_chunk0=lambda: a2a_head(1))
            proj_partial(1, wp1)
            a2a_head(2)
            proj_partial(2, wp2)

            # epilogue: bias + store
            for fc in range(3):
                for tcc in range(4):
                    ob = outp.tile([128, 512], F32, tag="ob")
                    nc.vector.tensor_tensor(
                        ob[:],
                        acc[fc * 4 + tcc][:],
                        bb_sb[:, fc * 512 : (fc + 1) * 512],
                        ALU.add,
                    )
                    nc.sync.dma_start(
                        out[tcc * 128 : (tcc + 1) * 128, fc * 512 : (fc + 1) * 512],
                        ob[:],
                    )
    nc.compile()
    return nc


_NC_CACHE = {}


def _get_nc():
    if "nc" not in _NC_CACHE:
        _NC_CACHE["nc"] = build_nc()
    return _NC_CACHE["nc"]


def make_in_maps(x, y, pos, y_token_weights, Wqkv, Wkv, q_norm_w, k_norm_w, Wproj, bproj):
    f = np.float32
    c32 = pos[:, :, 0].T
    s32 = pos[:, :, 1].T
    csT = np.ascontiguousarray(
        np.concatenate([c32, c32], 0).astype(ml_dtypes.bfloat16))   # [64, N]
    snT = np.ascontiguousarray(
        np.concatenate([-s32, s32], 0).astype(ml_dtypes.bfloat16))  # [64, N]
    wqs = (np.asarray(q_norm_w, dtype=f) * np.float32(HD) ** -0.5).reshape(1, HD)
    wkk = np.asarray(k_norm_w, dtype=f).reshape(1, HD)
    Wp = np.asarray(Wproj, dtype=f)
    # head-permuted Wproj: row block (t, j) = rows of head 3*j+t (same all cores)
    W = np.zeros((NHL, NQC, 128, C), dtype=f)
    for t in range(NHL):
        for j in range(NQC):
            h = 3 * j + t
            W[t, j] = Wp[h * 128 : (h + 1) * 128, :]
    wproj_perm = np.ascontiguousarray(
        W.reshape(NHL * NQC * 128, C).astype(ml_dtypes.bfloat16)
    )
    in_maps = []
    for c in range(8):
        b, g = c // 4, c % 4
        heads = [3 * g + i for i in range(NHL)]
        qcols = [Wqkv[:, h * HD : (h + 1) * HD] for h in heads]
        kcols = [Wqkv[:, C + h * HD : C + (h + 1) * HD] for h in heads]
        vcols = [Wqkv[:, 2 * C + h * HD : 2 * C + (h + 1) * HD] for h in heads]
        wqkv_c = np.ascontiguousarray(
            np.concatenate(qcols + kcols + vcols, axis=1), dtype=f
        )
        kcols2 = [Wkv[:, h * HD : (h + 1) * HD] for h in heads]
        vcols2 = [Wkv[:, C + h * HD : C + (h + 1) * HD] for h in heads]
        wkv_c = np.ascontiguousarray(np.concatenate(kcols2 + vcols2, axis=1), dtype=f)
        in_maps.append(
            {
                "xT": np.ascontiguousarray(np.asarray(x)[b].T.astype(ml_dtypes.bfloat16)),
                "yT": np.ascontiguousarray(np.asarray(y)[b].T.astype(ml_dtypes.bfloat16)),
                "wqkv": wqkv_c.astype(ml_dtypes.bfloat16),
                "wkv": wkv_c.astype(ml_dtypes.bfloat16),
                "wproj": wproj_perm,
                "wq": np.ascontiguousarray(wqs),
                "wk": np.ascontiguousarray(wkk),
                "cs": csT,
                "sn": snT,
                "ywT": np.ascontiguousarray(
                    np.asarray(y_token_weights)[b].reshape(M // 128, 128).T, dtype=f
                ),
                "bpr": np.asarray(bproj, dtype=f).reshape(1, C),
                "onesd": np.ones((128, 1), dtype=f),
                "onesb": np.ones((128, 1), dtype=ml_dtypes.bfloat16),
                "ones128": np.ones((1, 128), dtype=f),
                "m0d": np.full((128, 1), 1.0 if b == 0 else 0.0, dtype=f),
                "m1d": np.full((128, 1), 0.0 if b == 0 else 1.0, dtype=f),
            }
        )
    return in_maps


def kernel(x, y, pos, y_token_weights, Wqkv, Wkv, q_norm_w, k_norm_w, Wproj, bproj,
           _trace=False):
    x = np.asarray(x, dtype=np.float32)
    y = np.asarray(y, dtype=np.float32)
    pos = np.asarray(pos, dtype=np.float32)
    y_token_weights = np.asarray(y_token_weights, dtype=np.float32)
    nc = _get_nc()
    in_maps = make_in_maps(
        x, y, pos, y_token_weights,
        np.asarray(Wqkv), np.asarray(Wkv), np.asarray(q_norm_w),
        np.asarray(k_norm_w), np.asarray(Wproj), np.asarray(bproj),
    )
    res = run_bass_kernel_spmd(nc, in_maps, core_ids=list(range(8)), trace=_trace)
    outp = np.zeros((B, N, C), dtype=np.float32)
    for c in range(8):
        b, g = c // 4, c % 4
        outp[b, g * 512 : (g + 1) * 512, :] = res.results[c]["out"]
    if _trace:
        return outp, res
    return outp

